# revision 2
# baseline (speedup 1.0000x reference)
"""Trainium2 Bass kernel for nn_BiLinearMHSLayer.

Reference computation (per batch element b):
    t  = x @ fc_w.T + fc_b            [S, E]      (S=1024, IN=768, E=256)
    bl = (t @ bi_w.T).reshape(S,L,E) + bias       (L=12)
    out[i,l,j] = sum_e bl[i,l,e] * t[j,e]         [S, L, S]

Sharding: data-parallel over batch B=8 -> one batch element per NeuronCore.

Per-core dataflow (everything kept in "transposed" layout so the contraction
dim E lands on SBUF partitions for the PE-array matmuls):
    xT   [IN, S] = PE-transpose of x  (bf16, 48 128x128 tiles)
    tT   [E, S]  = fc_wT.T @ xT  + fc_b          (24 matmuls,  N=512)
    blT  [E*L,S] = bi_wT.T @ tT  + bias          (96 matmuls,  N=512)
    out  (per l) = blT_l.T @ tT                  (384 matmuls, N=512)
PSUM->SBUF evacuation is split across the Vector and Scalar engines; output
is staged in SBUF and DMA'd with 24KB-contiguous-per-partition descriptors.

Operands are cast to bf16 (accumulation is fp32 in PSUM); the |rel err|
vs the fp32 reference is ~1e-3 of max|out|.
"""

import json

import numpy as np

import concourse.bass as bass
import concourse.mybir as mybir
import concourse.tile as tile
from concourse.bass_utils import run_bass_kernel_spmd
from concourse.masks import make_identity

B, S, IN, E, L = 8, 1024, 768, 256, 12
N_CORES = 8
FP32 = mybir.dt.float32
BF16 = mybir.dt.bfloat16

# ---------------------------------------------------------------------------
# Workaround: walrus on this image rejects instructions carrying too many
# embedded sem waits ("Too many sync wait commands", CoreV3GenImpl
# setupSyncWait) -- notably the TileContext kernel-tail Drain.  Split excess
# waits onto EventSemaphore instructions inserted immediately before, on the
# same engine (identical semantics: the waits execute, in order, before the
# instruction).
_WAIT_CAPS = {}
_DEFAULT_WAIT_CAP = 1


def _fix_sync_waits(blob: bytes) -> bytes:
    j = json.loads(blob)
    n = 0
    for f in j.get("functions", []):
        for bb in f.get("blocks", []):
            out = []
            for inst in bb.get("instructions", []):
                si = inst.get("sync_info")
                waits = (si or {}).get("on_wait") or []
                cap = _WAIT_CAPS.get(inst.get("opcode"), _DEFAULT_WAIT_CAP)
                if len(waits) > cap:
                    excess, keep = waits[:-cap], waits[-cap:]
                    for w in excess:
                        n += 1
                        out.append({
                            "debug": inst.get("debug", 0),
                            "engine": inst["engine"],
                            "ins": [],
                            "name": f"waitsplit-{n}",
                            "opcode": "EventSemaphore",
                            "outs": [],
                            "sync_info": {"on_update": [], "on_wait": [w]},
                        })
                    si["on_wait"] = keep
                out.append(inst)
            bb["instructions"] = out
    return json.dumps(j).encode()


# ---------------------------------------------------------------------------
def _emit_body(nc, tc, pools, dram, copy_ctr):
    """Emit one full per-core computation. copy_ctr is a 1-elem list used to
    alternate PSUM-evacuation copies between the DVE and ACT engines."""
    x_d, fcw_d, fcb_d, biw_d, bias_d, out_d = dram
    (const_pool, big_pool, in_pool, psum_tr, psum_mm, stg_pool) = pools

    ident = const_pool.tile([128, 128], BF16, tag="ident")
    make_identity(nc, ident[:])

    def evac(dst_ap, src_ap, scalar=None):
        """PSUM -> SBUF copy (optionally + per-partition scalar add), engine
        round-robined 2:1 between DVE and ACT."""
        c = copy_ctr[0]
        copy_ctr[0] += 1
        if scalar is not None:
            # tensor_scalar lives on the vector-class engines only
            nc.vector.tensor_scalar_add(dst_ap, src_ap, scalar)
        elif c % 3 == 2:
            nc.scalar.copy(out=dst_ap, in_=src_ap)
        else:
            nc.vector.tensor_copy(dst_ap, src_ap)

    # ---- load + cast inputs -------------------------------------------------
    x_sb = in_pool.tile([128, 8 * 768], BF16, tag="x_sb")       # [s%128, (s/128, i)]
    for n in range(8):
        nc.gpsimd.dma_start(
            out=x_sb[:, n * 768:(n + 1) * 768], in_=x_d[n * 128:(n + 1) * 128, :])
    fcw_sb = in_pool.tile([128, 2 * 768], BF16, tag="fcw_sb")   # [e%128, (e/128, i)]
    for n in range(2):
        nc.gpsimd.dma_start(
            out=fcw_sb[:, n * 768:(n + 1) * 768], in_=fcw_d[n * 128:(n + 1) * 128, :])
    biw_sb = in_pool.tile([128, 24 * 256], BF16, tag="biw_sb")  # [f%128, (f/128, e)]
    for n in range(24):
        nc.gpsimd.dma_start(
            out=biw_sb[:, n * 256:(n + 1) * 256], in_=biw_d[n * 128:(n + 1) * 128, :])
    fcb_sb = const_pool.tile([128, 2], FP32, tag="fcb_sb")      # col ec: fc_b[ec*128+p]
    bias_sb = const_pool.tile([128, 2], FP32, tag="bias_sb")
    for c in range(2):
        nc.sync.dma_start(out=fcb_sb[:, c:c + 1], in_=fcb_d[c * 128:(c + 1) * 128, :])
        nc.sync.dma_start(out=bias_sb[:, c:c + 1], in_=bias_d[c * 128:(c + 1) * 128, :])

    # ---- transposes (PE identity-matmul, 128x128 bf16 tiles) ---------------
    xT = big_pool.tile([128, 6 * 1024], BF16, tag="xT")         # [i%128, (i/128, s)]
    fcwT = big_pool.tile([128, 6 * 256], BF16, tag="fcwT")      # [i%128, (i/128, e)]
    biwT = big_pool.tile([128, 2 * 3072], BF16, tag="biwT")     # [e%128, (e/128, f)]

    def pe_transpose(dst_ap, src_ap):
        p = psum_tr.tile([128, 128], BF16, tag="ptr")
        nc.tensor.transpose(p[:], src_ap, ident[:])
        evac(dst_ap, p[:])

    for ic in range(6):
        for n in range(2):   # fc_w blocks first: unblocks tT matmuls earliest
            pe_transpose(fcwT[:, ic * 256 + n * 128:ic * 256 + (n + 1) * 128],
                         fcw_sb[:, n * 768 + ic * 128:n * 768 + (ic + 1) * 128])
    for n in range(8):
        for ic in range(6):
            pe_transpose(xT[:, ic * 1024 + n * 128:ic * 1024 + (n + 1) * 128],
                         x_sb[:, n * 768 + ic * 128:n * 768 + (ic + 1) * 128])
    for ft in range(24):
        for kc in range(2):
            pe_transpose(biwT[:, kc * 3072 + ft * 128:kc * 3072 + (ft + 1) * 128],
                         biw_sb[:, ft * 256 + kc * 128:ft * 256 + (kc + 1) * 128])

    # ---- tT = fc_w @ x.T + fc_b   [E, S] ------------------------------------
    tT = big_pool.tile([128, 2 * 1024], BF16, tag="tT")         # [e%128, (e/128, s)]
    for ec in range(2):
        for ns in range(2):
            p = psum_mm.tile([128, 512], FP32, tag="pmm")
            for ic in range(6):
                nc.tensor.matmul(
                    p[:],
                    fcwT[:, ic * 256 + ec * 128:ic * 256 + (ec + 1) * 128],
                    xT[:, ic * 1024 + ns * 512:ic * 1024 + (ns + 1) * 512],
                    start=(ic == 0), stop=(ic == 5))
            evac(tT[:, ec * 1024 + ns * 512:ec * 1024 + (ns + 1) * 512],
                 p[:], scalar=fcb_sb[:, ec:ec + 1])

    # ---- blT = bi_w @ t.T + bias  [L*E, S] ----------------------------------
    blT = big_pool.tile([128, 24 * 1024], BF16, tag="blT")      # [f%128, (f/128, s)]
    for ft in range(24):
        for ns in range(2):
            p = psum_mm.tile([128, 512], FP32, tag="pmm")
            for kc in range(2):
                nc.tensor.matmul(
                    p[:],
                    biwT[:, kc * 3072 + ft * 128:kc * 3072 + (ft + 1) * 128],
                    tT[:, kc * 1024 + ns * 512:kc * 1024 + (ns + 1) * 512],
                    start=(kc == 0), stop=(kc == 1))
            evac(blT[:, ft * 1024 + ns * 512:ft * 1024 + (ns + 1) * 512],
                 p[:], scalar=bias_sb[:, ft % 2:ft % 2 + 1])

    # ---- scores: out[i, l, j] = sum_e blT[l*E+e, i] * tT[e, j] --------------
    out_v = out_d.rearrange("s l j -> s (l j)")                 # [1024, 12288]
    for it in range(8):
        for lh in range(2):
            stg = stg_pool.tile([128, 6 * 1024], FP32, tag="stg")
            for ll in range(6):
                l = lh * 6 + ll
                for jh in range(2):
                    p = psum_mm.tile([128, 512], FP32, tag="pmm")
                    for kc in range(2):
                        ft = 2 * l + kc
                        nc.tensor.matmul(
                            p[:],
                            blT[:, ft * 1024 + it * 128:ft * 1024 + (it + 1) * 128],
                            tT[:, kc * 1024 + jh * 512:kc * 1024 + (jh + 1) * 512],
                            start=(kc == 0), stop=(kc == 1))
                    evac(stg[:, ll * 1024 + jh * 512:ll * 1024 + (jh + 1) * 512], p[:])
            nc.sync.dma_start(
                out=out_v[it * 128:(it + 1) * 128, lh * 6144:(lh + 1) * 6144],
                in_=stg[:])


def build_nc(unroll: int = 1):
    """Build the Bass program.  unroll>1 repeats the whole body (for timing
    measurements via wall-clock differencing)."""
    nc = bass.Bass(trn_type="TRN2")
    x_d = nc.dram_tensor("x", [S, IN], FP32, kind="ExternalInput")
    fcw_d = nc.dram_tensor("fc_w", [E, IN], FP32, kind="ExternalInput")
    fcb_d = nc.dram_tensor("fc_b", [E, 1], FP32, kind="ExternalInput")
    biw_d = nc.dram_tensor("bi_w", [E * L, E], FP32, kind="ExternalInput")
    bias_d = nc.dram_tensor("bias", [E, 1], FP32, kind="ExternalInput")
    out_d = nc.dram_tensor("out", [S, L, S], FP32, kind="ExternalOutput")
    dram = (x_d, fcw_d, fcb_d, biw_d, bias_d, out_d)

    with tile.TileContext(nc) as tc:
        with (
            tc.tile_pool(name="const", bufs=1) as const_pool,
            tc.tile_pool(name="big", bufs=1) as big_pool,
            tc.tile_pool(name="inp", bufs=1) as in_pool,
            tc.tile_pool(name="psum_tr", bufs=2, space="PSUM") as psum_tr,
            tc.tile_pool(name="psum_mm", bufs=6, space="PSUM") as psum_mm,
            tc.tile_pool(name="stg", bufs=3) as stg_pool,
        ):
            pools = (const_pool, big_pool, in_pool, psum_tr, psum_mm, stg_pool)
            copy_ctr = [0]
            for _ in range(unroll):
                _emit_body(nc, tc, pools, dram, copy_ctr)

    blob = _fix_sync_waits(nc.to_json_bytes())
    nc.to_json_bytes = lambda: blob
    return nc


_CACHE = {}


def _get_nc(unroll: int = 1):
    if unroll not in _CACHE:
        _CACHE[unroll] = build_nc(unroll)
    return _CACHE[unroll]


def kernel(input_tensor, fc_w, fc_b, bi_w, bias):
    input_tensor = np.ascontiguousarray(np.asarray(input_tensor, dtype=np.float32))
    fc_w = np.ascontiguousarray(np.asarray(fc_w, dtype=np.float32))
    fc_b = np.ascontiguousarray(np.asarray(fc_b, dtype=np.float32)).reshape(E, 1)
    bi_w = np.ascontiguousarray(np.asarray(bi_w, dtype=np.float32))
    bias = np.ascontiguousarray(np.asarray(bias, dtype=np.float32)).reshape(E, 1)
    assert input_tensor.shape == (B, S, IN)

    nc = _get_nc()
    in_maps = [
        {"x": input_tensor[c], "fc_w": fc_w, "fc_b": fc_b, "bi_w": bi_w, "bias": bias}
        for c in range(N_CORES)
    ]
    res = run_bass_kernel_spmd(nc, in_maps, core_ids=list(range(N_CORES)))
    return np.stack([res.results[c]["out"] for c in range(N_CORES)], axis=0)


# revision 32
# speedup vs baseline: 398.1033x; 398.1033x over previous
"""Trainium2 Bass kernel for nn_BiLinearMHSLayer.

Reference computation (per batch element b):
    t  = x @ fc_w.T + fc_b            [S, E]      (S=1024, IN=768, E=256)
    bl = (t @ bi_w.T).reshape(S,L,E) + bias       (L=12)
    out[i,l,j] = sum_e bl[i,l,e] * t[j,e]         [S, L, S]

Sharding: data-parallel over batch B=8 -> one batch element per NeuronCore.

Per-core dataflow (everything kept in "transposed" layout so the contraction
dim lands on SBUF partitions for the PE-array matmuls):
    xT   [IN, S] = PE-transpose of x  (bf16, 48 128x128 tiles)
    tT   [E, S]  = fc_wT.T @ xT  + fc_b          (24 matmuls,  N=512)
    blT  [E*L,S] = bi_wT.T @ tT  + bias          (96 matmuls,  N=512)
    out  (per l) = blT_l.T @ tT                  (384 matmuls, N=512)

The schedule is software-pipelined over S-halves: the first half of xT/tT/blT
unblocks score matmuls + output DMA for i-tiles 0-3 / j-half 0 while the
second half is still being produced, so the 50MB/core output write (the
roofline term) starts early.  PSUM->SBUF evacuation alternates between the
Vector and Scalar engines.  Operands are cast to bf16 (fp32 accumulation in
PSUM); |err| vs the fp32 reference is ~4e-3 of max|out|.
"""

import json

import numpy as np

import concourse.bass as bass
import concourse.mybir as mybir
import concourse.tile as tile
from concourse.bass_utils import run_bass_kernel_spmd
from concourse.masks import make_identity

B, S, IN, E, L = 8, 1024, 768, 256, 12
N_CORES = 8
FP32 = mybir.dt.float32
BF16 = mybir.dt.bfloat16
ACT_COPY = mybir.ActivationFunctionType.Copy
ACT_IDENT = mybir.ActivationFunctionType.Identity

# ---------------------------------------------------------------------------
# Workaround: walrus on this image rejects instructions carrying more than one
# embedded sem wait ("Too many sync wait commands", CoreV3GenImpl
# setupSyncWait).  Split excess waits onto EventSemaphore instructions
# inserted immediately before, on the same engine (identical semantics: the
# waits execute, in order, before the instruction).
_WAIT_CAPS = {}
_DEFAULT_WAIT_CAP = 1


def _fix_sync_waits(blob: bytes) -> bytes:
    j = json.loads(blob)
    n = 0
    for f in j.get("functions", []):
        for bb in f.get("blocks", []):
            out = []
            for inst in bb.get("instructions", []):
                si = inst.get("sync_info")
                waits = (si or {}).get("on_wait") or []
                cap = _WAIT_CAPS.get(inst.get("opcode"), _DEFAULT_WAIT_CAP)
                if len(waits) > cap:
                    excess, keep = waits[:-cap], waits[-cap:]
                    for w in excess:
                        n += 1
                        out.append({
                            "debug": inst.get("debug", 0),
                            "engine": inst["engine"],
                            "ins": [],
                            "name": f"waitsplit-{n}",
                            "opcode": "EventSemaphore",
                            "outs": [],
                            "sync_info": {"on_update": [], "on_wait": [w]},
                        })
                    si["on_wait"] = keep
                out.append(inst)
            bb["instructions"] = out
    return json.dumps(j).encode()


# ---------------------------------------------------------------------------
_DMA_SRC_CONST = False  # debug ablation: output DMAs read a constant tile
_EVAC_MOD = 3           # 1 of every _EVAC_MOD evacuations goes to ACT
_FUSE_LH = False        # True: one 3.1MB DMA per (i-tile, j-half) unit


def _emit_consts(nc, const_pool):
    ident = const_pool.tile([128, 128], BF16, tag="ident")
    make_identity(nc, ident[:])
    stg_const = None
    if _DMA_SRC_CONST:
        stg_const = const_pool.tile([128, 6 * 512], FP32, tag="stg_const")
        nc.vector.memset(stg_const[:], 1.0)
    return ident, stg_const


def _emit_body(nc, tc, pools, dram, ctr, consts):
    """Emit one full per-core computation."""
    x_d, fcw_d, fcb_d, biw_d, bias_d, out_d = dram
    (const_pool, big_pool, in_pool, psum_tr, psum_mm, stg_pool, dram_pool) = pools
    ident, stg_const = consts

    def evac(dst_ap, src_ap, bias_ap=None):
        """PSUM -> SBUF copy (+ optional per-partition bias add).  Split 2:1
        between DVE and ACT (ACT's per-element copy rate is ~2x slower)."""
        c = ctr[0]
        ctr[0] += 1
        if c % _EVAC_MOD != _EVAC_MOD - 1:
            if bias_ap is not None:
                nc.vector.tensor_scalar_add(dst_ap, src_ap, bias_ap)
            else:
                nc.vector.tensor_copy(dst_ap, src_ap)
        elif bias_ap is not None:
            # Copy doesn't accept an AP bias; Identity does.
            nc.scalar.activation(dst_ap, src_ap, ACT_IDENT, bias=bias_ap)
        else:
            nc.scalar.activation(dst_ap, src_ap, ACT_COPY)

    # ---- persistent SBUF tensors -------------------------------------------
    x_sb = in_pool.tile([128, 8 * 768], BF16, tag="x_sb")       # [s%128, (s/128, i)]
    fcw_sb = in_pool.tile([128, 2 * 768], BF16, tag="fcw_sb")   # [e%128, (e/128, i)]
    fcb_sb = const_pool.tile([128, 2], FP32, tag="fcb_sb")      # col ec: fc_b[ec*128+p]
    bias_sb = const_pool.tile([128, 2], FP32, tag="bias_sb")
    xT = big_pool.tile([128, 6 * 1024], BF16, tag="xT")         # [i%128, (i/128, s)]
    fcwT = big_pool.tile([128, 6 * 256], BF16, tag="fcwT")      # [i%128, (i/128, e)]
    biwT = big_pool.tile([128, 2 * 3072], BF16, tag="biwT")     # [e%128, (e/128, f)]
    tT = big_pool.tile([128, 2 * 1024], BF16, tag="tT")         # [e%128, (e/128, s)]
    blT = big_pool.tile([128, 24 * 1024], BF16, tag="blT")      # [f%128, (f/128, s)]
    biw_sb = in_pool.tile([128, 24 * 256], BF16, tag="biw_sb")  # [f%128, (f/128, e)]

    # ---- input loads (SWDGE cast-DMA fp32->bf16, merged) -------------------
    # Order = startup critical path: fc_w (gates first PE transposes), x half
    # 0 (gates xT/tT), bi_w (gates biwT -> blT), then the rest.
    x_src = x_d.rearrange("(n p) i -> p n i", p=128)            # [128, 8, 768]
    x_dst = x_sb[:].rearrange("p (n i) -> p n i", n=8)
    nc.gpsimd.dma_start(
        out=fcw_sb[:].rearrange("p (n i) -> p n i", n=2),
        in_=fcw_d.rearrange("(n p) i -> p n i", p=128))
    nc.gpsimd.dma_start(out=x_dst[:, 0:4, :], in_=x_src[:, 0:4, :])
    nc.gpsimd.dma_start(
        out=biw_sb[:].rearrange("p (n e) -> p n e", n=24),
        in_=biw_d.rearrange("(n p) e -> p n e", p=128))
    nc.gpsimd.dma_start(out=x_dst[:, 4:8, :], in_=x_src[:, 4:8, :])
    for c in range(2):
        nc.sync.dma_start(out=fcb_sb[:, c:c + 1], in_=fcb_d[c * 128:(c + 1) * 128, :])
        nc.sync.dma_start(out=bias_sb[:, c:c + 1], in_=bias_d[c * 128:(c + 1) * 128, :])

    # ---- building blocks ----------------------------------------------------
    def pe_transpose_group(dst_ap, srcs):
        """Transpose len(srcs) 128x128 blocks into one PSUM bank, evacuate
        with a single wide copy. dst_ap free size must be len(srcs)*128 and
        column-ordered to match srcs."""
        p = psum_tr.tile([128, 512], BF16, tag="ptr")
        for g, src in enumerate(srcs):
            nc.tensor.transpose(p[:, g * 128:(g + 1) * 128], src, ident[:])
        evac(dst_ap, p[:, 0:len(srcs) * 128])

    def emit_fcwT():
        for ic0 in range(0, 6, 2):
            pe_transpose_group(
                fcwT[:, ic0 * 256:(ic0 + 2) * 256],
                [fcw_sb[:, n * 768 + ic * 128:n * 768 + (ic + 1) * 128]
                 for ic in (ic0, ic0 + 1) for n in (0, 1)])

    def emit_xT(nh):
        # xT columns ic*1024 + n*128 for the 4 s-tiles n of half nh
        for ic in range(6):
            pe_transpose_group(
                xT[:, ic * 1024 + nh * 512:ic * 1024 + (nh + 1) * 512],
                [x_sb[:, n * 768 + ic * 128:n * 768 + (ic + 1) * 128]
                 for n in range(nh * 4, nh * 4 + 4)])

    def emit_biwT():
        # biwT columns kc*3072 + ft*128; group 4 consecutive ft per bank
        for kc in range(2):
            for ft0 in range(0, 24, 4):
                pe_transpose_group(
                    biwT[:, kc * 3072 + ft0 * 128:kc * 3072 + (ft0 + 4) * 128],
                    [biw_sb[:, ft * 256 + kc * 128:ft * 256 + (kc + 1) * 128]
                     for ft in range(ft0, ft0 + 4)])

    def emit_tT(ns):
        for ec in range(2):
            p = psum_mm.tile([128, 512], FP32, tag="pmm")
            for ic in range(6):
                nc.tensor.matmul(
                    p[:],
                    fcwT[:, ic * 256 + ec * 128:ic * 256 + (ec + 1) * 128],
                    xT[:, ic * 1024 + ns * 512:ic * 1024 + (ns + 1) * 512],
                    start=(ic == 0), stop=(ic == 5))
            evac(tT[:, ec * 1024 + ns * 512:ec * 1024 + (ns + 1) * 512],
                 p[:], bias_ap=fcb_sb[:, ec:ec + 1])

    def emit_blT(c0, w):
        # one w-wide column sub-block (s in [c0, c0+w)) for all 24 f-tiles
        for ft in range(24):
            p = psum_mm.tile([128, 512], FP32, tag="pmm")
            for kc in range(2):
                nc.tensor.matmul(
                    p[:, 0:w],
                    biwT[:, kc * 3072 + ft * 128:kc * 3072 + (ft + 1) * 128],
                    tT[:, kc * 1024 + c0:kc * 1024 + c0 + w],
                    start=(kc == 0), stop=(kc == 1))
            evac(blT[:, ft * 1024 + c0:ft * 1024 + c0 + w],
                 p[:, 0:w], bias_ap=bias_sb[:, ft % 2:ft % 2 + 1])

    def emit_wave(its, jh):
        # output unit = (i-tile, j-half, l-half): [128 i, 6 l, 512 j].
        # Score matmuls for an l-pair land in one 2-bank PSUM tile so each
        # evacuation is a single [128,1024] copy; each staged half-unit is
        # DMA'd independently (finer rotation hides DMA completion latency).
        for it in its:
            if _FUSE_LH:
                stg = stg_pool.tile([128, L * 512], FP32, tag="stg")
            for lh in range(2):
                if not _FUSE_LH:
                    stg = stg_pool.tile([128, 6 * 512], FP32, tag="stg")
                for lp in range(3):
                    p = psum_mm.tile([128, 1024], FP32, tag="pmm")
                    for g in range(2):
                        l = lh * 6 + lp * 2 + g
                        for kc in range(2):
                            ft = 2 * l + kc
                            nc.tensor.matmul(
                                p[:, g * 512:(g + 1) * 512],
                                blT[:, ft * 1024 + it * 128:ft * 1024 + (it + 1) * 128],
                                tT[:, kc * 1024 + jh * 512:kc * 1024 + (jh + 1) * 512],
                                start=(kc == 0), stop=(kc == 1))
                    off = (lh * 6 + lp * 2) * 512 if _FUSE_LH else lp * 1024
                    evac(stg[:, off:off + 1024], p[:])
                if not _FUSE_LH:
                    src = stg_const if _DMA_SRC_CONST else stg
                    nc.sync.dma_start(
                        out=out_d[it * 128:(it + 1) * 128, lh * 6:lh * 6 + 6,
                                  jh * 512:(jh + 1) * 512],
                        in_=src[:].rearrange("p (l j) -> p l j", l=6))
            if _FUSE_LH:
                src = stg_const if _DMA_SRC_CONST else stg
                nc.sync.dma_start(
                    out=out_d[it * 128:(it + 1) * 128, :, jh * 512:(jh + 1) * 512],
                    in_=src[:].rearrange("p (l j) -> p l j", l=L))

    # ---- schedule -----------------------------------------------------------
    # blT n-block 0 covers i-tiles 0-3, n-block 1 covers 4-7; tT n-block jh
    # is the j-half.  Waves are ordered so the output DMA stream starts as
    # early as possible and never starves.
    emit_fcwT()
    emit_xT(0)
    emit_tT(0)
    emit_biwT()
    emit_blT(0, 256)
    emit_wave((0, 1), 0)
    emit_blT(256, 256)
    emit_wave((2, 3), 0)
    emit_xT(1)
    emit_tT(1)
    emit_wave((0, 1), 1)
    emit_wave((2, 3), 1)
    emit_blT(512, 512)
    emit_wave((4, 5, 6, 7), 0)
    emit_wave((4, 5, 6, 7), 1)


def build_nc(unroll: int = 1):
    """Build the Bass program.  unroll>1 repeats the whole body (for timing
    measurements via wall-clock differencing)."""
    nc = bass.Bass(trn_type="TRN2")
    x_d = nc.dram_tensor("x", [S, IN], FP32, kind="ExternalInput")
    fcw_d = nc.dram_tensor("fc_w", [E, IN], FP32, kind="ExternalInput")
    fcb_d = nc.dram_tensor("fc_b", [E, 1], FP32, kind="ExternalInput")
    biw_d = nc.dram_tensor("bi_w", [E * L, E], FP32, kind="ExternalInput")
    bias_d = nc.dram_tensor("bias", [E, 1], FP32, kind="ExternalInput")
    out_d = nc.dram_tensor("out", [S, L, S], FP32, kind="ExternalOutput")
    dram = (x_d, fcw_d, fcb_d, biw_d, bias_d, out_d)

    with tile.TileContext(nc) as tc:
        with (
            tc.tile_pool(name="const", bufs=1) as const_pool,
            tc.tile_pool(name="big", bufs=1) as big_pool,
            tc.tile_pool(name="inp", bufs=1) as in_pool,
            tc.tile_pool(name="psum_tr", bufs=2, space="PSUM") as psum_tr,
            tc.tile_pool(name="psum_mm", bufs=3, space="PSUM") as psum_mm,
            tc.tile_pool(name="stg", bufs=3 if _FUSE_LH else 6) as stg_pool,
            tc.tile_pool(name="dram", bufs=1, space="DRAM") as dram_pool,
        ):
            pools = (const_pool, big_pool, in_pool, psum_tr, psum_mm, stg_pool,
                     dram_pool)
            ctr = [0]
            consts = _emit_consts(nc, const_pool)
            for _ in range(unroll):
                _emit_body(nc, tc, pools, dram, ctr, consts)

    blob = _fix_sync_waits(nc.to_json_bytes())
    nc.to_json_bytes = lambda: blob
    return nc


_CACHE = {}


def _get_nc(unroll: int = 1):
    if unroll not in _CACHE:
        _CACHE[unroll] = build_nc(unroll)
    return _CACHE[unroll]


def kernel(input_tensor, fc_w, fc_b, bi_w, bias):
    input_tensor = np.ascontiguousarray(np.asarray(input_tensor, dtype=np.float32))
    fc_w = np.ascontiguousarray(np.asarray(fc_w, dtype=np.float32))
    fc_b = np.ascontiguousarray(np.asarray(fc_b, dtype=np.float32)).reshape(E, 1)
    bi_w = np.ascontiguousarray(np.asarray(bi_w, dtype=np.float32))
    bias = np.ascontiguousarray(np.asarray(bias, dtype=np.float32)).reshape(E, 1)
    assert input_tensor.shape == (B, S, IN)

    nc = _get_nc()
    in_maps = [
        {"x": input_tensor[c], "fc_w": fc_w, "fc_b": fc_b, "bi_w": bi_w, "bias": bias}
        for c in range(N_CORES)
    ]
    res = run_bass_kernel_spmd(nc, in_maps, core_ids=list(range(N_CORES)))
    return np.stack([res.results[c]["out"] for c in range(N_CORES)], axis=0)
